# revision 15
# baseline (speedup 1.0000x reference)
"""ConvLSTM regression kernel for Trainium2 (8 NeuronCores, data-parallel).

Reference computation (B=32, T=50, S=2048, C=8, F=32, K=2, STRIDE=2):
  xc = conv1d(x, kernel, stride=2, SAME) + bias          # per (b,t): (1024, 128)
  scan over t: hc = conv1d(h, rec_kernel, stride=1, SAME)
               z  = xc[t] + hc;  i,f,c~,o gates -> LSTM update
  out = flatten(h_T) @ dense_w + dense_b                 # (B, 1)

Sharding: batch 32 -> 4 per core, weights replicated, no collectives.

Device-side design (per core), raw Bass with manual semaphores (TileContext's
auto-sync emits multi-wait instructions this container's walrus rejects):
  - One fused matmul per 128-position tile: stationary lhsT (81, 128) =
    [h(32); h_shift(32); x_pairs(16); ones(1)] slice of a staging tensor,
    moving rhs = W (81, 128).  PSUM (128 positions, 128 gates) -> full-lane
    elementwise.
  - Gate affine (0.2z+0.5) folded into W and its bias row; gate order [i,f,o,c].
  - h feedback pos-major -> f-major via XBAR transposing DMA into the stage;
    +1 spatial shift is a plain DMA copy at col offset (per-batch col 1023
    stays 0 forever).
  - fp16 matmul operands (1 PE cycle/row), fp32 elementwise state.
"""

import numpy as np

B, T, S, C, F = 32, 50, 2048, 8, 32
NCORES = 8
BL = B // NCORES          # 4 batch items per core
SH = S // 2               # 1024 spatial positions after stride-2 conv
G = 4 * F                 # 128 gates
KC = 81                   # contraction: 32 h + 32 h_shift + 16 x + 1 ones
NPOS = BL * SH            # 4096 positions per step per core
NT = NPOS // 128          # 32 position tiles
HALF = NT // 2            # 16 tiles per half

_CACHE = {}


def _build_nc(t_steps=T):
    import concourse.bass as bass
    import concourse.mybir as mybir

    dt = mybir.dt
    Alu = mybir.AluOpType
    Act = mybir.ActivationFunctionType

    nc = bass.Bass()
    xw_d = nc.declare_dram_parameter("xw", [T, 16, BL, SH], dt.float16, isOutput=False)
    wg_d = nc.declare_dram_parameter("wg", [KC, G], dt.float16, isOutput=False)
    out_d = nc.declare_dram_parameter("hout", [128, NT * F], dt.float32, isOutput=True)

    TS = t_steps

    from contextlib import ExitStack
    es = ExitStack()
    with es:
        stage = es.enter_context(nc.sbuf_tensor("stage", [KC, NPOS], dt.float16))
        wg_sb = es.enter_context(nc.sbuf_tensor("wg_sb", [KC, G], dt.float16))
        h_pm = es.enter_context(nc.sbuf_tensor("h_pm", [128, NT * 128], dt.float16))
        h_fm = es.enter_context(nc.sbuf_tensor("h_fm", [128, NPOS], dt.float16))
        c_st = es.enter_context(nc.sbuf_tensor("c_st", [128, NT * F], dt.float32))
        h32 = es.enter_context(nc.sbuf_tensor("h32", [128, NT * F], dt.float32))
        hs0 = es.enter_context(nc.sbuf_tensor("hs0", [128, HALF * 96], dt.float32))
        hs1 = es.enter_context(nc.sbuf_tensor("hs1", [128, HALF * 96], dt.float32))
        tct0 = es.enter_context(nc.sbuf_tensor("tct0", [128, HALF * F], dt.float32))
        tct1 = es.enter_context(nc.sbuf_tensor("tct1", [128, HALF * F], dt.float32))
        tht0 = es.enter_context(nc.sbuf_tensor("tht0", [128, HALF * F], dt.float32))
        tht1 = es.enter_context(nc.sbuf_tensor("tht1", [128, HALF * F], dt.float32))
        tmp0 = es.enter_context(nc.sbuf_tensor("tmp0", [128, HALF * F], dt.float32))
        tmp1 = es.enter_context(nc.sbuf_tensor("tmp1", [128, HALF * F], dt.float32))
        ps0 = es.enter_context(nc.psum_tensor("ps0", [128, HALF * G], dt.float32))
        ps1 = es.enter_context(nc.psum_tensor("ps1", [128, HALF * G], dt.float32))
        sx = es.enter_context(nc.semaphore("sx"))
        str_ = es.enter_context(nc.semaphore("str_"))
        spe = es.enter_context(nc.semaphore("spe"))
        sdve = es.enter_context(nc.semaphore("sdve"))
        sact = es.enter_context(nc.semaphore("sact"))
        sh32 = es.enter_context(nc.semaphore("sh32"))
        block = es.enter_context(nc.Block())
        PS = [ps0, ps1]
        HS = [hs0, hs1]
        TCT = [tct0, tct1]
        THT = [tht0, tht1]
        TMP = [tmp0, tmp1]

        def hsv(Hh):  # (128, HALF, 96)
            return HS[Hh].rearrange("p (t g) -> p t g", g=96)

        def zv(Hh):  # (128, HALF, G) psum view
            return PS[Hh].rearrange("p (t g) -> p t g", g=G)

        def cvw(Hh):  # c-state half view (128, HALF, F)
            return c_st.rearrange("p (t f) -> p t f", f=F)[
                :, Hh * HALF : (Hh + 1) * HALF, :
            ]

        @block.sync
        def _(sync):
            sync.dma_start(out=wg_sb[:, :], in_=wg_d[:, :]).then_inc(sx, 16)
            sync.wait_ge(sdve, 4)  # memsets done before x(0) overwrites rows 64-79
            sync.dma_start(
                out=stage[64:80, :].rearrange("r (b w) -> r b w", b=BL),
                in_=xw_d[0],
            ).then_inc(sx, 16)
            for t in range(TS):
                if t < TS - 1:
                    # XBAR transpose writes full 128-partition tiles, so it
                    # must land in its own 128-high tensor, not inside stage.
                    sync.wait_ge(sdve, 4 + 12 * t + 12)  # h_pm ready
                    sync.wait_ge(str_, 48 * t)           # t-1 copies done reading h_fm
                    sync.dma_start(
                        out=h_fm[:, :].rearrange("f (j p) -> f j p", p=128),
                        in_=h_pm[:, :],
                        transpose=True,
                    ).then_inc(str_, 16)
                    sync.wait_ge(spe, 2 * t + 2)         # step-t MMs done with stage
                    sync.wait_ge(str_, 48 * t + 16)      # transpose done
                    sync.dma_start(
                        out=stage[0:32, :], in_=h_fm[0:32, :]
                    ).then_inc(str_, 16)
                    sync.dma_start(
                        out=stage[32:64, :].rearrange("f (b w) -> f b w", b=BL)[
                            :, :, 0:1023
                        ],
                        in_=h_fm[0:32, :].rearrange("f (b w) -> f b w", b=BL)[
                            :, :, 1:1024
                        ],
                    ).then_inc(str_, 16)
                    sync.dma_start(
                        out=stage[64:80, :].rearrange("r (b w) -> r b w", b=BL),
                        in_=xw_d[t + 1],
                    ).then_inc(sx, 16)
            sync.wait_ge(sh32, 2)  # h32 ops done
            sync.dma_start(out=out_d[:, :], in_=h32[:, :]).then_inc(sx, 16)
            sync.wait_ge(sx, 16 * (TS + 2))

        @block.tensor
        def _(tensor):
            for t in range(TS):
                for Hh in range(2):
                    if Hh == 0:
                        tensor.wait_ge(sx, 16 * (2 + t))
                        if t > 0:
                            tensor.wait_ge(str_, 48 * t)
                        else:
                            tensor.wait_ge(sdve, 4)
                    if t > 0:
                        # psum[Hh] free: step t-1 readers done
                        tensor.wait_ge(sdve, 4 + 12 * (t - 1) + 6 * Hh + 1)
                        tensor.wait_ge(sact, 6 * (t - 1) + 3 * Hh + 2)
                    for j16 in range(HALF):
                        j = Hh * HALF + j16
                        mm = nc.tensor.matmul(
                            PS[Hh][:, j16 * G : (j16 + 1) * G],
                            stage[:, j * 128 : (j + 1) * 128],
                            wg_sb[:, :],
                            start=True,
                            stop=True,
                        )
                        if j16 == HALF - 1:
                            mm.then_inc(spe, 1)

        @block.vector
        def _(vector):
            nc.vector.memset(stage[0:64, :], 0.0).then_inc(sdve, 1)
            nc.vector.memset(stage[64:81, :], 1.0).then_inc(sdve, 1)
            nc.vector.memset(c_st[:, :], 0.0).then_inc(sdve, 1)
            nc.vector.memset(h_pm[:, :], 0.0).then_inc(sdve, 1)
            for t in range(TS):
                for Hh in range(2):
                    hs = hsv(Hh)
                    z = zv(Hh)
                    ch = cvw(Hh)
                    tct = TCT[Hh].rearrange("p (t f) -> p t f", f=F)
                    tht = THT[Hh].rearrange("p (t f) -> p t f", f=F)
                    tmp = TMP[Hh].rearrange("p (t f) -> p t f", f=F)
                    vector.wait_ge(spe, 2 * t + Hh + 1)
                    nc.vector.tensor_scalar(
                        hs[:, :, 64:96], z[:, :, 64:96], 1.0, 0.0, Alu.min, Alu.max
                    ).then_inc(sdve, 1)
                    vector.wait_ge(sact, 6 * t + 3 * Hh + 1)
                    nc.vector.tensor_scalar_min(
                        hs[:, :, 0:64], hs[:, :, 0:64], 1.0
                    ).then_inc(sdve, 1)
                    vector.wait_ge(sact, 6 * t + 3 * Hh + 2)
                    nc.vector.drain()  # flush min_if (and t=0 memsets) before reads
                    nc.vector.tensor_tensor(
                        tmp[:, :, :], hs[:, :, 0:32], tct[:, :, :], Alu.mult
                    ).then_inc(sdve, 1)
                    nc.vector.tensor_tensor(
                        ch, hs[:, :, 32:64], ch, Alu.mult
                    ).then_inc(sdve, 1)
                    nc.vector.drain()  # flush tmp/cmul before cadd
                    nc.vector.tensor_tensor(
                        ch, ch, tmp[:, :, :], Alu.add
                    ).then_inc(sdve, 1)
                    vector.wait_ge(sact, 6 * t + 3 * Hh + 3)
                    hv = h_pm.rearrange("p (j x) -> p j x", x=128)[
                        :, Hh * HALF : (Hh + 1) * HALF, 0:F
                    ]
                    nc.vector.tensor_tensor(
                        hv, hs[:, :, 64:96], tht[:, :, :], Alu.mult
                    ).then_inc(sdve, 1)
                    if t == TS - 1:
                        h32v = h32.rearrange("p (t f) -> p t f", f=F)[
                            :, Hh * HALF : (Hh + 1) * HALF, :
                        ]
                        nc.vector.tensor_tensor(
                            h32v, hs[:, :, 64:96], tht[:, :, :], Alu.mult
                        ).then_inc(sh32, 1)

        @block.scalar
        def _(scalar):
            for t in range(TS):
                for Hh in range(2):
                    hs = hsv(Hh)
                    z = zv(Hh)
                    ch = cvw(Hh)
                    tct = TCT[Hh].rearrange("p (t f) -> p t f", f=F)
                    tht = THT[Hh].rearrange("p (t f) -> p t f", f=F)
                    scalar.wait_ge(spe, 2 * t + Hh + 1)
                    if t > 0:
                        # hs/tct/tht free: step t-1 half-Hh consumers done
                        scalar.wait_ge(sdve, 4 + 12 * (t - 1) + 6 * Hh + 6)
                    nc.scalar.activation(
                        hs[:, :, 0:64], z[:, :, 0:64], Act.Relu
                    ).then_inc(sact, 1)
                    nc.scalar.activation(
                        tct[:, :, :], z[:, :, 96:128], Act.Tanh
                    ).then_inc(sact, 1)
                    scalar.wait_ge(sdve, 4 + 12 * t + 6 * Hh + 5)
                    nc.scalar.activation(
                        tht[:, :, :], ch, Act.Tanh
                    ).then_inc(sact, 1)

    return nc


def _host_prep(x, kernel, rec_kernel, bias):
    """Build xw (T, 16, B, SH) fp16 and wg (81, 128) fp16 host-side."""
    # x: (B, T, S, C) -> (T, 2C, B, SH) with row e*8+c = x[., ., 2w+e, c]
    xw = (
        x.reshape(B, T, SH, 2, C)
        .transpose(1, 3, 4, 0, 2)
        .reshape(T, 2 * C, B, SH)
        .astype(np.float16)
    )
    # gate reorder [i, f, c~, o] -> [i, f, o, c~]
    perm = np.concatenate([np.arange(0, 64), np.arange(96, 128), np.arange(64, 96)])
    wg = np.zeros((KC, G), np.float32)
    wg[0:32] = rec_kernel[0][:, perm]
    wg[32:64] = rec_kernel[1][:, perm]
    wg[64:72] = kernel[0][:, perm]  # rows 64 + c  (e=0)
    wg[72:80] = kernel[1][:, perm]  # rows 72 + c  (e=1)
    bp = bias[perm]
    wg[:, 0:96] *= 0.2
    wg[80, 0:96] = 0.2 * bp[0:96] + 0.5
    wg[80, 96:128] = bp[96:128]
    return xw, wg.astype(np.float16)


def kernel(x, kernel, rec_kernel, bias, dense_w, dense_b):
    from concourse.bass_utils import run_bass_kernel_spmd

    x = np.asarray(x, np.float32)
    kernel = np.asarray(kernel, np.float32)
    rec_kernel = np.asarray(rec_kernel, np.float32)
    bias = np.asarray(bias, np.float32)
    dense_w = np.asarray(dense_w, np.float32)
    dense_b = np.asarray(dense_b, np.float32)

    xw, wg = _host_prep(x, kernel, rec_kernel, bias)

    if "nc" not in _CACHE:
        _CACHE["nc"] = _build_nc()
    nc = _CACHE["nc"]

    in_maps = [
        {"xw": np.ascontiguousarray(xw[:, :, ci * BL : (ci + 1) * BL]), "wg": wg}
        for ci in range(NCORES)
    ]
    res = run_bass_kernel_spmd(nc, in_maps, core_ids=list(range(NCORES))).results

    out = np.zeros((B, 1), np.float32)
    # hout[p, 32*(b*8 + jp) + f] = h[b, 128*jp + p, f]
    dw = dense_w.reshape(SH, F)
    for ci in range(NCORES):
        h = np.asarray(res[ci]["hout"], np.float32)  # (128, 1024)
        h = h.reshape(128, BL, 8, F)                  # p, b, jp, f
        h = h.transpose(1, 2, 0, 3).reshape(BL, SH, F)
        out[ci * BL : (ci + 1) * BL, 0] = np.einsum("bwf,wf->b", h, dw) + dense_b[0]
    return out
